# revision 1
# baseline (speedup 1.0000x reference)
#!/usr/bin/env python3
"""2-layer GAT on 8 NeuronCores (Bass/Tile) — v2 blocked design.

Sharding: global dst chunks (128 nodes) are size-sorted and dealt to
(slot, core) so the 8 chunks sharing a slot have near-equal edge counts
(SPMD static shapes pad to the max over cores).  Node features, the L1
table (tab1) and the L2 table (tab2) all live in the SAME assignment
layout (row = core*NPAD + slot*128 + dl), so one edge-index stream
serves both layers and source windows always line up.

Phases C/E process blocks of B=7 slots: one dma_gather per
(block, window), one indicator / attention / exp op per block, matmuls
accumulate per-slot psums packed 2-per-bank, epilogues batched per
block, log-softmax + OUT write once at the end.
"""
import sys
import numpy as np

sys.path.insert(0, "/opt/pypackages")
sys.path.insert(0, "/opt/trn_rl_repo")

import concourse.bass as bass
import concourse.bacc as bacc
import concourse.tile as tile
import concourse.mybir as mybir
from concourse.bass_utils import run_bass_kernel_spmd

# problem constants
N = 100000
F_IN = 512
NHID = 16
HEADS = 8
NCLASS = 40
NEG_SLOPE = 0.2

NCORES = 8
DCH = 128                    # dsts per chunk/slot
NSLOT = 98                   # slots per core
GCH = NSLOT * NCORES         # 784 global chunk slots (782 real + 2 pad)
NPAD = NSLOT * DCH           # 12544 rows per core (nodes, assignment order)
NSCH = 4
SCHW = 2 * NPAD              # 25088 table rows per window (cores {2w,2w+1})
B = 7                        # slots per block
NBLK = NSLOT // B            # 14 blocks

ROW1 = 256    # f16 elems per L1 table row (512B): [h1 128 | asrc 8 | pad]
G1W = 136     # f16 elems gathered per L1 row (272B)
ROW2 = 128    # f16 elems per L2 table row (256B): [h2 40 | one | asrc2 | pad]
G2W = 42
ROWA = 128    # f16 elems per adst-table row (256B)

F16 = mybir.dt.float16
F32 = mybir.dt.float32
I16 = mybir.dt.int16


def _wrap(v):
    """Wrap an int16 stream (len % 16 == 0) into the dma_gather idx layout
    [16, L/16], replicated to 128 partitions."""
    w = v.reshape(-1, 16).T
    return np.tile(w, (8, 1))


def _dma_gather_raw(gp, out_ap, in_ap, idxs_ap, num_idxs, elem_size, elem_step,
                    queue_num=0):
    """dma_gather allowing elem_size (elems read per row) that is not a
    multiple of 256B; the table row stride (elem_step) still must be."""
    from concourse.bass import exact_div
    stride_bytes = elem_step * mybir.dt.size(in_ap.dtype)
    stride_bytes_256 = exact_div(stride_bytes, 256)
    _in_ap = gp.lower_ap_dma(in_ap, for_custom_bir_dma=True)
    _idxs_ap = gp.lower_ap(idxs_ap)
    _out_ap = gp.lower_ap(out_ap)
    return gp.add_instruction(
        mybir.InstDMAGatherAnt(
            name=gp.bass.get_next_instruction_name(),
            ins=[*_in_ap, _idxs_ap, gp.lower_val_access(gp.to_reg(num_idxs))],
            outs=[_out_ap],
            transpose=False, num_idxs=num_idxs, elem_size=elem_size,
            stride_bytes_256=stride_bytes_256, gen_mode=0,
            single_packet=False, queue_num=queue_num,
            sbuf_tokens_per_rank=0, sbuf_free_dim_per_rank=0,
            sbuf_free_dim_pad_per_rank=0, sbuf_byte_offset=0))


def _prep(x, edge_index, W1, att_src1, att_dst1, W2, att_src2, att_dst2,
          b1=None, b2=None):
    """Host-side sharding/packing. Returns (in_maps, meta)."""
    # self-loops are handled analytically in the epilogues (no gather), so
    # the edge stream carries only the real edges
    x = np.asarray(x, np.float32)
    src = np.asarray(edge_index[0]).astype(np.int64)
    dst = np.asarray(edge_index[1]).astype(np.int64)

    # --- sorted chunk -> (slot, core) assignment ---------------------------
    gch_d = dst // DCH                            # 0..781
    sizes = np.bincount(gch_d, minlength=GCH)
    order_ch = np.argsort(-sizes, kind="stable")  # descending size
    asn_core = np.empty(GCH, np.int64)
    asn_slot = np.empty(GCH, np.int64)
    rr = np.arange(GCH)
    asn_slot[order_ch] = rr // NCORES
    asn_core[order_ch] = rr % NCORES

    core_e = asn_core[gch_d]                      # dst-owner core per edge
    slot_e = asn_slot[gch_d]
    dl_e = dst % DCH

    # --- source position in the (shared) assignment-layout table -----------
    gch_s = src // DCH
    sc = asn_core[gch_s]
    sp = sc * NPAD + asn_slot[gch_s] * DCH + (src % DCH)
    sch = sp // SCHW                              # window = src core pair
    sloc = sp - sch * SCHW                        # 0..25087, int16-safe

    # --- cells: (dst core, block, window, slot-in-block), pad each to 128 --
    blk = slot_e // B
    sib = slot_e % B
    cell = ((core_e * NBLK + blk) * NSCH + sch) * B + sib
    npc_cells = NBLK * NSCH * B                   # cells per core
    order = np.argsort(cell * (SCHW + 1) + sloc, kind="stable")
    cell_s = cell[order]
    counts = np.bincount(cell_s, minlength=NCORES * npc_cells)
    shapes = (np.ceil(counts.reshape(NCORES, npc_cells).max(axis=0)
                      / 16.0) * 16).astype(np.int64)        # [npc_cells]
    # pad each (blk, window) run of B cells to a multiple of 128 by
    # inflating the run's last cell
    shp4 = shapes.reshape(NBLK, NSCH, B)
    shp4[:, :, B - 1] += (-shp4.sum(axis=2)) % 128
    shapes = shp4.reshape(-1)
    cell_starts = np.concatenate([[0], np.cumsum(shapes)])
    L = int(cell_starts[-1])
    t_total = L // 128

    # slab table: one (tile, cell) pair per tile a cell overlaps.  Shared
    # across cores (derived from shapes alone).  Slabs sorted by (tile, cell).
    pairs = []
    for c in range(npc_cells):
        cs, ce = int(cell_starts[c]), int(cell_starts[c + 1])
        for t in range(cs // 128, (ce + 127) // 128):
            pairs.append((t, c))
    pairs.sort()
    slab_of = {}
    slab_tile = np.empty(len(pairs), np.int64)
    slab_cell = np.empty(len(pairs), np.int64)
    for i, (t, c) in enumerate(pairs):
        slab_of[(c, t)] = i
        slab_tile[i] = t
        slab_cell[i] = c
    nslab = len(pairs)

    group_start = np.concatenate([[0], np.cumsum(counts)])
    rank = np.arange(len(cell_s)) - group_start[cell_s]
    pos = cell_starts[cell_s % npc_cells] + rank
    core_s = cell_s // npc_cells

    IDX1 = np.zeros((NCORES, L), np.int16)
    IDXD = np.zeros((NCORES, L), np.int16)
    IDX1[core_s, pos] = sloc[order].astype(np.int16)
    IDXD[core_s, pos] = (slot_e * DCH + dl_e)[order].astype(np.int16)

    # dstloc stream indexed by SLAB: edges of (cell c, tile t) land in slab
    # slab_of[(c, t)]; other slabs covering tile t keep 255 at that partition
    cell_local_e = cell_s % npc_cells
    tile_e = pos // 128
    slab_e = np.array([slab_of[(int(c), int(t))]
                       for c, t in zip(cell_local_e, tile_e)], np.int64)
    DSTL = np.full((NCORES, nslab * 128), 255.0, np.float16)
    DSTL[core_s, slab_e * 128 + pos % 128] = dl_e[order].astype(np.float16)

    IDX1w = np.stack([_wrap(IDX1[k]) for k in range(NCORES)])
    IDXDw = np.stack([_wrap(IDXD[k]) for k in range(NCORES)])
    DSTLw = DSTL.reshape(NCORES, nslab, 128).transpose(0, 2, 1).copy()

    # --- weights -----------------------------------------------------------
    asrc1 = np.asarray(att_src1).reshape(HEADS, NHID)
    adst1 = np.asarray(att_dst1).reshape(HEADS, NHID)
    W1r = np.asarray(W1).reshape(F_IN, HEADS, NHID)
    W1as = np.einsum("khc,hc->kh", W1r, asrc1)
    W1ad = np.einsum("khc,hc->kh", W1r, adst1)
    W1ext = np.concatenate([np.asarray(W1), W1as, W1ad],
                           axis=1).astype(np.float16)          # [512, 144]
    W2as = np.asarray(W2) @ np.asarray(att_src2).reshape(NCLASS, 1)
    W2ad = np.asarray(W2) @ np.asarray(att_dst2).reshape(NCLASS, 1)
    W2ext = np.concatenate([np.asarray(W2), W2as, W2ad],
                           axis=1).astype(np.float16)          # [128, 42]

    iota = np.broadcast_to(np.arange(128, dtype=np.float16), (128, 128)).copy()

    # --- per-core node features in assignment order ------------------------
    xpad = np.zeros((GCH * DCH, F_IN), np.float16)
    xpad[:N] = x.astype(np.float16)
    in_maps = []
    for k in range(NCORES):
        chunks = order_ch[np.arange(NSLOT) * NCORES + k]       # slot -> chunk
        xs = xpad.reshape(GCH, DCH, F_IN)[chunks]              # [98,128,512]
        xT = np.ascontiguousarray(
            xs.reshape(NPAD, F_IN).T)                          # [512, 12544]
        in_maps.append({
            "xT": xT, "W1ext": W1ext, "W2ext": W2ext,
            "IDX1": IDX1w[k], "IDXD": IDXDw[k], "DSTLOC": DSTLw[k],
            "iota": iota,
            "B1": (np.zeros((1, 128), np.float32) if b1 is None
                   else np.asarray(b1, np.float32).reshape(1, 128)),
            "B2": (np.zeros((1, NCLASS), np.float32) if b2 is None
                   else np.asarray(b2, np.float32).reshape(1, NCLASS)),
        })
    meta = {"shapes": shapes.reshape(NBLK, NSCH, B),
            "t_total": t_total, "order_ch": order_ch,
            "nslab": nslab, "slab_tile": slab_tile, "slab_cell": slab_cell,
            "cell_starts": cell_starts}
    return in_maps, meta


def _build(meta, phases="ABCDE", clevel=9):
    from concourse.masks import make_identity

    shp = meta["shapes"]            # [NBLK, NSCH, B] padded edge counts
    t_total = meta["t_total"]
    blk_tiles = [int(shp[b].sum()) // 128 for b in range(NBLK)]
    TMAX = max(blk_tiles)
    # slab structure: per block, list of (slot j, slab-local idx, tile-local)
    nslab = int(meta["nslab"])
    slab_tile = meta["slab_tile"]
    slab_cell = meta["slab_cell"]
    blk_of = slab_cell // (NSCH * B)
    slab_off = [int(np.searchsorted(blk_of, b)) for b in range(NBLK + 1)]
    tile_off = np.concatenate([[0], np.cumsum(blk_tiles)])
    blk_mm = []
    for b in range(NBLK):
        lo, hi = slab_off[b], slab_off[b + 1]
        blk_mm.append([(int(slab_cell[i]) % B, i - lo,
                        int(slab_tile[i]) - int(tile_off[b]))
                       for i in range(lo, hi)])
    blk_ns = [slab_off[b + 1] - slab_off[b] for b in range(NBLK)]
    NSMAX = max(blk_ns)
    meta_rt = (blk_mm, blk_ns, slab_off, NSMAX)

    nc = bacc.Bacc("TRN2", target_bir_lowering=False, debug=False,
                   enable_asserts=False, num_devices=NCORES,
                   num_swdge_queues=4)

    xT = nc.dram_tensor("xT", [F_IN, NPAD], F16, kind="ExternalInput")
    W1e = nc.dram_tensor("W1ext", [F_IN, 144], F16, kind="ExternalInput")
    W2e = nc.dram_tensor("W2ext", [128, 42], F16, kind="ExternalInput")
    IDX1 = nc.dram_tensor("IDX1", [128, t_total * 8], I16, kind="ExternalInput")
    IDXD = nc.dram_tensor("IDXD", [128, t_total * 8], I16, kind="ExternalInput")
    DSTLOC = nc.dram_tensor("DSTLOC", [128, nslab], F16, kind="ExternalInput")
    IOTA = nc.dram_tensor("iota", [128, 128], F16, kind="ExternalInput")
    B1 = nc.dram_tensor("B1", [1, 128], F32, kind="ExternalInput")
    B2 = nc.dram_tensor("B2", [1, NCLASS], F32, kind="ExternalInput")
    OUT = nc.dram_tensor("out", [NPAD, NCLASS], F32, kind="ExternalOutput")

    tab1_sh = nc.dram_tensor("tab1_sh", [NPAD, ROW1], F16, kind="Internal")
    tab1 = nc.dram_tensor("tab1", [NPAD * NCORES, ROW1], F16, kind="Internal",
                          addr_space="Shared")
    tab2_sh = nc.dram_tensor("tab2_sh", [NPAD, ROW2], F16, kind="Internal")
    tab2 = nc.dram_tensor("tab2", [NPAD * NCORES, ROW2], F16, kind="Internal",
                          addr_space="Shared")
    adr1 = nc.dram_tensor("adr1", [NPAD, ROWA], F16, kind="Internal")
    adr2 = nc.dram_tensor("adr2", [NPAD, ROWA], F16, kind="Internal")

    with tile.TileContext(nc) as tc:
        if "A" in phases:
            _phase_a(nc, tc, xT, W1e, tab1_sh, adr1)
        if "B" in phases:
            nc.gpsimd.collective_compute(
                "AllGather", mybir.AluOpType.bypass,
                replica_groups=[list(range(NCORES))],
                ins=[tab1_sh[:]], outs=[tab1[:]])
        if "C" in phases:
            _phase_c(nc, tc, shp, blk_tiles, TMAX, make_identity,
                     IDX1, IDXD, DSTLOC, IOTA, B1, W2e, tab1, tab1_sh, adr1,
                     tab2_sh, adr2, clevel, meta_rt)
        if "D" in phases:
            nc.gpsimd.collective_compute(
                "AllGather", mybir.AluOpType.bypass,
                replica_groups=[list(range(NCORES))],
                ins=[tab2_sh[:]], outs=[tab2[:]])
        if "E" in phases:
            _phase_e(nc, tc, shp, blk_tiles, TMAX,
                     IDX1, IDXD, DSTLOC, IOTA, B2, tab2, tab2_sh, adr2, OUT,
                     clevel, meta_rt)
        else:
            with tc.tile_pool(name="sbZ", bufs=1) as sbz:
                z = sbz.tile([128, NSLOT * 40], F32, tag="z", name="z")
                nc.vector.memset(z[:], 0.0)
                nc.sync.dma_start(
                    OUT[:].rearrange("(j p) e -> p j e", p=128),
                    z[:].rearrange("p (j e) -> p j e", e=40))

    nc.compile()
    return nc


def _phase_a(nc, tc, xT, W1e, tab1_sh, adr1):
    """h1 = x @ W1ext per 128-node slot; writes tab1_sh ([h1|asrc]) and
    adr1 (adst), batched 7 slots per DMA."""
    AG = 7
    with tc.tile_pool(name="sbA", bufs=1) as sba, \
         tc.tile_pool(name="sbA2", bufs=3) as sba2, \
         tc.tile_pool(name="psA", bufs=4, space="PSUM") as psa:
        xts = [sba.tile([128, NPAD], F16, tag=f"xt{k}", name=f"xt{k}")
               for k in range(4)]
        w1s = [sba.tile([128, 144], F16, tag=f"w1{k}", name=f"w1{k}")
               for k in range(4)]
        for k in range(4):
            nc.sync.dma_start(xts[k][:], xT[k * 128:(k + 1) * 128, :])
            nc.sync.dma_start(w1s[k][:], W1e[k * 128:(k + 1) * 128, :])
        for gb in range(NSLOT // AG):
            row = sba2.tile([128, AG * 136], F16, tag="rowA", name="rowA")
            t8 = sba2.tile([128, AG * 8], F16, tag="t8A", name="t8A")
            for j in range(AG):
                nt = gb * AG + j
                ps = psa.tile([128, 144], F32, tag="psA", name="psA")
                for k in range(4):
                    nc.tensor.matmul(ps[:],
                                     lhsT=xts[k][:, nt * 128:(nt + 1) * 128],
                                     rhs=w1s[k][:], start=(k == 0),
                                     stop=(k == 3))
                nc.scalar.copy(row[:, j * 136:(j + 1) * 136], ps[:, 0:136])
                nc.scalar.copy(t8[:, j * 8:(j + 1) * 8], ps[:, 136:144])
            nc.sync.dma_start(
                tab1_sh[gb * AG * 128:(gb + 1) * AG * 128, 0:136]
                .rearrange("(g p) e -> p g e", p=128),
                row[:].rearrange("p (g e) -> p g e", e=136))
            nc.sync.dma_start(
                adr1[gb * AG * 128:(gb + 1) * AG * 128, 0:8]
                .rearrange("(g p) e -> p g e", p=128),
                t8[:].rearrange("p (g e) -> p g e", e=8))


def _phase_c(nc, tc, shp, blk_tiles, TMAX, make_identity,
             IDX1, IDXD, DSTLOC, IOTA, B1, W2e, tab1, tab1_sh, adr1,
             tab2_sh, adr2, clevel=9, meta_rt=None):
    blk_mm, blk_ns, slab_off, NSMAX = meta_rt
    eq = mybir.AluOpType.is_equal
    mult = mybir.AluOpType.mult
    amax = mybir.AluOpType.max
    aadd = mybir.AluOpType.add
    AF = mybir.ActivationFunctionType

    with tc.tile_pool(name="sbC", bufs=1) as sbc, \
         tc.tile_pool(name="sbC2", bufs=2) as sb2, \
         tc.tile_pool(name="sbC3", bufs=2) as sb3, \
         tc.tile_pool(name="psC", bufs=1, space="PSUM") as psc, \
         tc.tile_pool(name="psC2", bufs=2, space="PSUM") as psc2:
        iot = sbc.tile([128, 128], F16, tag="iota", name="iotc")
        nc.sync.dma_start(iot[:], IOTA[:])
        ident = sbc.tile([128, 128], F16, tag="ident", name="ident")
        make_identity(nc, ident[:])
        w2s = sbc.tile([128, 42], F16, tag="w2s", name="w2s")
        nc.sync.dma_start(w2s[:], W2e[:])
        b1t = sbc.tile([128, 128], F32, tag="b1t", name="b1t")
        nc.sync.dma_start(b1t[:], B1[:].to_broadcast([128, 128]))

        off = 0
        for b in range(NBLK):
            T = blk_tiles[b]
            i1 = sb2.tile([128, TMAX * 8], I16, tag="i1", name="i1")
            nc.sync.dma_start(i1[:, 0:T * 8], IDX1[:, off * 8:(off + T) * 8])
            idd = sb2.tile([128, TMAX * 8], I16, tag="idd", name="idd")
            nc.sync.dma_start(idd[:, 0:T * 8], IDXD[:, off * 8:(off + T) * 8])
            NS = blk_ns[b]
            dlc = sb2.tile([128, NSMAX], F16, tag="dlc", name="dlc")
            nc.sync.dma_start(dlc[:, 0:NS],
                              DSTLOC[:, slab_off[b]:slab_off[b] + NS])

            g1 = sb2.tile([128, TMAX * G1W], F16, tag="g1", name="g1")
            coff = 0
            for s in range(NSCH):
                cl = int(shp[b, s].sum())
                if cl == 0:
                    continue
                for p0 in range(coff, coff + cl, 8192):
                    pl = min(8192, coff + cl - p0)
                    _dma_gather_raw(
                        nc.gpsimd,
                        g1[:, (p0 // 128) * G1W:((p0 + pl) // 128) * G1W]
                        .rearrange("p (t e) -> p t e", e=G1W),
                        tab1[s * SCHW:(s + 1) * SCHW, :],
                        i1[:, p0 // 16:(p0 + pl) // 16], pl, G1W, ROW1,
                        queue_num=s)
                coff += cl
            nedge = T * 128
            ga = sb2.tile([128, TMAX * 8], F16, tag="ga", name="ga")
            for p0 in range(0, nedge, 8192):
                pl = min(8192, nedge - p0)
                _dma_gather_raw(
                    nc.gpsimd,
                    ga[:, (p0 // 128) * 8:((p0 + pl) // 128) * 8]
                    .rearrange("p (t e) -> p t e", e=8),
                    adr1[:], idd[:, p0 // 16:(p0 + pl) // 16], pl, 8, ROWA,
                    queue_num=(b + p0 // 8192) % 4)

            if clevel < 2:
                off += T
                continue
            g13 = g1[:, 0:T * G1W].rearrange("p (t e) -> p t e", e=G1W)
            ga3 = ga[:, 0:T * 8].rearrange("p (t e) -> p t e", e=8)

            # attention weight w = exp(leaky_relu(asrc[src] + adst[dst]));
            # exp lands in g1's asrc slot (becomes the denominator columns)
            att = sb2.tile([128, TMAX * 8], F16, tag="att", name="att")
            at3 = att[:, 0:T * 8].rearrange("p (t h) -> p t h", h=8)
            nc.vector.tensor_tensor(out=at3, in0=g13[:, :, 128:136],
                                    in1=ga3[:, :, 0:8], op=aadd)
            nc.vector.scalar_tensor_tensor(
                out=at3, in0=at3, scalar=NEG_SLOPE, in1=at3,
                op0=mult, op1=amax)
            nc.scalar.activation(
                out=g13[:, :, 128:136], in_=at3, func=AF.Exp)
            # weight features in place: g1[:,:,0:128] *= w (broadcast / head)
            nc.vector.tensor_tensor(
                out=g13[:, :, 0:128].rearrange("p t (h c) -> p t h c", c=NHID),
                in0=g13[:, :, 0:128].rearrange("p t (h c) -> p t h c", c=NHID),
                in1=g13[:, :, 128:136].rearrange("p t (h c) -> p t h c", c=1)
                .to_broadcast([128, T, 8, NHID]),
                op=mult)

            if clevel < 3:
                off += T
                continue
            # indicator one-hot per (edge, slab)
            ind = sb2.tile([128, NSMAX * 128], F16, tag="ind", name="ind")
            nc.vector.tensor_tensor(
                out=ind[:, 0:NS * 128].rearrange("p (t s) -> p t s", s=128),
                in0=iot[:].rearrange("p (t s) -> p t s", t=1)
                .to_broadcast([128, NS, 128]),
                in1=dlc[:, 0:NS].rearrange("p (t s) -> p t s", s=1)
                .to_broadcast([128, NS, 128]),
                op=eq)

            # segment sums into per-slot psums (2 slots per bank)
            pss = [psc.tile([128, 272], F32, tag=f"pss{j}", name=f"pss{j}")
                   for j in range((B + 1) // 2)]
            for ps in pss:
                nc.vector.memset(ps[:], 0.0)
            mm = blk_mm[b]
            last_of = {}
            for i, (j, sl, tl) in enumerate(mm):
                last_of[j] = i
            for i, (j, sl, tl) in enumerate(mm):
                ps = pss[j // 2]
                nc.tensor.matmul(
                    ps[:, (j % 2) * 136:(j % 2) * 136 + 136],
                    lhsT=ind[:, sl * 128:(sl + 1) * 128],
                    rhs=g1[:, tl * G1W:tl * G1W + 136],
                    start=False, stop=(last_of[j] == i))

            if clevel < 4:
                off += T
                continue
            # ---- epilogue, batched over the block ----
            # self-loop contribution: w_self * h1[d] added locally (no gather)
            h1l = sb3.tile([128, B * 136], F16, tag="h1l", name="h1l")
            nc.sync.dma_start(
                h1l[:].rearrange("p (j e) -> p j e", e=136),
                tab1_sh[b * B * 128:(b + 1) * B * 128, 0:136]
                .rearrange("(j p) e -> p j e", p=128))
            aa = sb3.tile([128, B * 8], F16, tag="aa", name="aa")
            nc.sync.dma_start(
                aa[:].rearrange("p (j e) -> p j e", e=8),
                adr1[b * B * 128:(b + 1) * B * 128, 0:8]
                .rearrange("(j p) e -> p j e", p=128))
            h1l3 = h1l[:].rearrange("p (j e) -> p j e", e=136)
            ats = sb3.tile([128, B * 8], F16, tag="ats", name="ats")
            ats3 = ats[:].rearrange("p (j h) -> p j h", h=8)
            nc.vector.tensor_tensor(out=ats3, in0=h1l3[:, :, 128:136],
                                    in1=aa[:].rearrange("p (j h) -> p j h", h=8),
                                    op=aadd)
            nc.vector.scalar_tensor_tensor(
                out=ats3, in0=ats3, scalar=NEG_SLOPE, in1=ats3,
                op0=mult, op1=amax)
            ws = sb3.tile([128, B * 8], F16, tag="ws", name="ws")
            nc.scalar.activation(out=ws[:], in_=ats[:], func=AF.Exp)
            slw = sb3.tile([128, B * 128], F16, tag="slw", name="slw")
            nc.vector.tensor_tensor(
                out=slw[:].rearrange("p (j h c) -> p j h c", h=8, c=NHID),
                in0=h1l3[:, :, 0:128].rearrange("p j (h c) -> p j h c", c=NHID),
                in1=ws[:].rearrange("p (j h c) -> p j h c", h=8, c=1)
                .to_broadcast([128, B, 8, NHID]),
                op=mult)

            ob = sb3.tile([128, B * 136], F32, tag="ob", name="ob")
            for j in range(B):
                nc.scalar.copy(ob[:, j * 136:(j + 1) * 136],
                               pss[j // 2][:, (j % 2) * 136:(j % 2) * 136 + 136])
            ob3 = ob[:].rearrange("p (j e) -> p j e", e=136)
            nc.vector.tensor_tensor(
                out=ob3[:, :, 0:128], in0=ob3[:, :, 0:128],
                in1=slw[:].rearrange("p (j e) -> p j e", e=128), op=aadd)
            nc.vector.tensor_tensor(
                out=ob3[:, :, 128:136], in0=ob3[:, :, 128:136],
                in1=ws[:].rearrange("p (j h) -> p j h", h=8), op=aadd)
            rc = sb3.tile([128, B * 8], F32, tag="rc", name="rc")
            nc.vector.reciprocal(rc[:].rearrange("p (j h) -> p j h", h=8),
                                 ob3[:, :, 128:136])
            o1 = sb3.tile([128, B * 128], F32, tag="o1", name="o1")
            nc.vector.tensor_tensor(
                out=o1[:].rearrange("p (j h c) -> p j h c", h=8, c=NHID),
                in0=ob3[:, :, 0:128].rearrange("p j (h c) -> p j h c", c=NHID),
                in1=rc[:].rearrange("p (j h c) -> p j h c", h=8, c=1)
                .to_broadcast([128, B, 8, NHID]),
                op=mult)
            nc.vector.tensor_tensor(
                out=o1[:].rearrange("p (j e) -> p j e", e=128),
                in0=o1[:].rearrange("p (j e) -> p j e", e=128),
                in1=b1t[:].rearrange("p (j e) -> p j e", j=1)
                .to_broadcast([128, B, 128]), op=aadd)
            # elu = max(x,0) + exp(min(x,0)) - 1   (fp16 exp path)
            t1 = sb3.tile([128, B * 128], F16, tag="t1", name="t1")
            nc.vector.tensor_scalar_min(t1[:], o1[:], 0.0)
            t2 = sb3.tile([128, B * 128], F16, tag="t2", name="t2")
            nc.scalar.activation(out=t2[:], in_=t1[:], func=AF.Exp)
            nc.vector.tensor_scalar_add(t2[:], t2[:], -1.0)
            nc.vector.tensor_scalar_max(o1[:], o1[:], 0.0)
            elu = sb3.tile([128, B * 128], F16, tag="elu", name="elu")
            nc.vector.tensor_tensor(out=elu[:], in0=o1[:], in1=t2[:], op=aadd)

            if clevel < 5:
                off += T
                continue
            # layer-2 rows: h2 = eluT @ W2ext per slot
            h2 = sb3.tile([128, B * 42], F16, tag="h2", name="h2")
            a2 = sb3.tile([128, B * 8], F16, tag="a2", name="a2")
            for j in range(B):
                psT = psc2.tile([128, 128], F16, tag="psT", name="psT")
                nc.tensor.transpose(psT[:], elu[:, j * 128:(j + 1) * 128],
                                    ident[:])
                eluT = sb3.tile([128, 128], F16, tag="eluT", name="eluT")
                nc.scalar.copy(eluT[:], psT[:])
                ps2a = psc2.tile([128, 42], F32, tag="ps2a", name="ps2a")
                nc.tensor.matmul(ps2a[:], lhsT=eluT[:], rhs=w2s[:],
                                 start=True, stop=True)
                nc.vector.tensor_copy(h2[:, j * 42:j * 42 + NCLASS],
                                      ps2a[:, 0:NCLASS])
                nc.vector.memset(h2[:, j * 42 + NCLASS:j * 42 + NCLASS + 1],
                                 1.0)
                nc.vector.tensor_copy(h2[:, j * 42 + 41:j * 42 + 42],
                                      ps2a[:, NCLASS:NCLASS + 1])
                nc.vector.tensor_copy(
                    a2[:, j * 8:(j + 1) * 8].rearrange("p (r h) -> p r h", h=1),
                    ps2a[:, 41:42].rearrange("p (r h) -> p r h", r=1)
                    .to_broadcast([128, 8, 1]))
            nc.sync.dma_start(
                tab2_sh[b * B * 128:(b + 1) * B * 128, 0:42]
                .rearrange("(j p) e -> p j e", p=128),
                h2[:].rearrange("p (j e) -> p j e", e=42))
            nc.sync.dma_start(
                adr2[b * B * 128:(b + 1) * B * 128, 0:8]
                .rearrange("(j p) e -> p j e", p=128),
                a2[:].rearrange("p (j e) -> p j e", e=8))
            off += T


def _phase_e(nc, tc, shp, blk_tiles, TMAX,
             IDX1, IDXD, DSTLOC, IOTA, B2, tab2, tab2_sh, adr2, OUT,
             clevel=9, meta_rt=None):
    blk_mm, blk_ns, slab_off, NSMAX = meta_rt
    eq = mybir.AluOpType.is_equal
    mult = mybir.AluOpType.mult
    amax = mybir.AluOpType.max
    aadd = mybir.AluOpType.add
    sub = mybir.AluOpType.subtract
    AF = mybir.ActivationFunctionType
    AX = mybir.AxisListType

    with tc.tile_pool(name="sbE", bufs=1) as sbe, \
         tc.tile_pool(name="sbE2", bufs=2) as se2, \
         tc.tile_pool(name="psE", bufs=2, space="PSUM") as pse:
        iot = sbe.tile([128, 128], F16, tag="iotaE", name="iote")
        nc.sync.dma_start(iot[:], IOTA[:])
        b2t = sbe.tile([128, NCLASS], F32, tag="b2t", name="b2t")
        nc.sync.dma_start(b2t[:], B2[:].to_broadcast([128, NCLASS]))
        lacc = sbe.tile([128, NSLOT * 41], F32, tag="lacc", name="lacc")

        off = 0
        for b in range(NBLK):
            T = blk_tiles[b]
            i1 = se2.tile([128, TMAX * 8], I16, tag="i1e", name="i1e")
            nc.sync.dma_start(i1[:, 0:T * 8], IDX1[:, off * 8:(off + T) * 8])
            idd = se2.tile([128, TMAX * 8], I16, tag="idde", name="idde")
            nc.sync.dma_start(idd[:, 0:T * 8], IDXD[:, off * 8:(off + T) * 8])
            NS = blk_ns[b]
            dlc = se2.tile([128, NSMAX], F16, tag="dlce", name="dlce")
            nc.sync.dma_start(dlc[:, 0:NS],
                              DSTLOC[:, slab_off[b]:slab_off[b] + NS])

            g2 = se2.tile([128, TMAX * G2W], F16, tag="g2", name="g2")
            coff = 0
            for s in range(NSCH):
                cl = int(shp[b, s].sum())
                if cl == 0:
                    continue
                for p0 in range(coff, coff + cl, 8192):
                    pl = min(8192, coff + cl - p0)
                    _dma_gather_raw(
                        nc.gpsimd,
                        g2[:, (p0 // 128) * G2W:((p0 + pl) // 128) * G2W]
                        .rearrange("p (t e) -> p t e", e=G2W),
                        tab2[s * SCHW:(s + 1) * SCHW, :],
                        i1[:, p0 // 16:(p0 + pl) // 16], pl, G2W, ROW2,
                        queue_num=s)
                coff += cl
            nedge = T * 128
            ga2 = se2.tile([128, TMAX * 8], F16, tag="ga2", name="ga2")
            for p0 in range(0, nedge, 8192):
                pl = min(8192, nedge - p0)
                _dma_gather_raw(
                    nc.gpsimd,
                    ga2[:, (p0 // 128) * 8:((p0 + pl) // 128) * 8]
                    .rearrange("p (t e) -> p t e", e=8),
                    adr2[:], idd[:, p0 // 16:(p0 + pl) // 16], pl, 8, ROWA,
                    queue_num=(b + p0 // 8192) % 4)

            if clevel < 6:
                nc.vector.memset(lacc[:, b * B * 41:(b + 1) * B * 41], 0.0)
                off += T
                continue
            g23 = g2[:, 0:T * G2W].rearrange("p (t e) -> p t e", e=G2W)
            ga23 = ga2[:, 0:T * 8].rearrange("p (t e) -> p t e", e=8)

            at2 = se2.tile([128, TMAX], F16, tag="at2", name="at2")
            at23 = at2[:, 0:T].rearrange("p (t h) -> p t h", h=1)
            nc.vector.tensor_tensor(out=at23, in0=g23[:, :, 41:42],
                                    in1=ga23[:, :, 0:1], op=aadd)
            nc.vector.scalar_tensor_tensor(
                out=at23, in0=at23, scalar=NEG_SLOPE, in1=at23,
                op0=mult, op1=amax)
            w2t = se2.tile([128, TMAX], F16, tag="w2t", name="w2t")
            nc.scalar.activation(out=w2t[:, 0:T], in_=at2[:, 0:T], func=AF.Exp)
            nc.vector.tensor_tensor(
                out=g23[:, :, 0:41],
                in0=g23[:, :, 0:41],
                in1=w2t[:, 0:T].rearrange("p (t s) -> p t s", s=1)
                .to_broadcast([128, T, 41]),
                op=mult)

            if clevel < 7:
                nc.vector.memset(lacc[:, b * B * 41:(b + 1) * B * 41], 0.0)
                off += T
                continue
            ind = se2.tile([128, NSMAX * 128], F16, tag="inde", name="inde")
            nc.vector.tensor_tensor(
                out=ind[:, 0:NS * 128].rearrange("p (t s) -> p t s", s=128),
                in0=iot[:].rearrange("p (t s) -> p t s", t=1)
                .to_broadcast([128, NS, 128]),
                in1=dlc[:, 0:NS].rearrange("p (t s) -> p t s", s=1)
                .to_broadcast([128, NS, 128]),
                op=eq)

            pse_t = pse.tile([128, B * 41], F32, tag="psE", name="psE")
            nc.vector.memset(pse_t[:], 0.0)
            mm = blk_mm[b]
            last_of = {}
            for i, (j, sl, tl) in enumerate(mm):
                last_of[j] = i
            for i, (j, sl, tl) in enumerate(mm):
                nc.tensor.matmul(
                    pse_t[:, j * 41:(j + 1) * 41],
                    lhsT=ind[:, sl * 128:(sl + 1) * 128],
                    rhs=g2[:, tl * G2W:tl * G2W + 41],
                    start=False, stop=(last_of[j] == i))
            if clevel < 8:
                nc.scalar.copy(lacc[:, b * B * 41:(b + 1) * B * 41],
                               pse_t[:, 0:B * 41])
                off += T
                continue
            # self-loop contribution (local rows of tab2_sh / adr2)
            h2l = se2.tile([128, B * 42], F16, tag="h2l", name="h2l")
            nc.sync.dma_start(
                h2l[:].rearrange("p (j e) -> p j e", e=42),
                tab2_sh[b * B * 128:(b + 1) * B * 128, 0:42]
                .rearrange("(j p) e -> p j e", p=128))
            a2l = se2.tile([128, B * 8], F16, tag="a2l", name="a2l")
            nc.sync.dma_start(
                a2l[:].rearrange("p (j e) -> p j e", e=8),
                adr2[b * B * 128:(b + 1) * B * 128, 0:8]
                .rearrange("(j p) e -> p j e", p=128))
            h2l3 = h2l[:].rearrange("p (j e) -> p j e", e=42)
            ats2 = se2.tile([128, B], F16, tag="ats2", name="ats2")
            a23 = ats2[:].rearrange("p (j h) -> p j h", h=1)
            nc.vector.tensor_tensor(
                out=a23, in0=h2l3[:, :, 41:42],
                in1=a2l[:].rearrange("p (j h) -> p j h", h=8)[:, :, 0:1],
                op=aadd)
            nc.vector.scalar_tensor_tensor(
                out=a23, in0=a23, scalar=NEG_SLOPE, in1=a23,
                op0=mult, op1=amax)
            ws2 = se2.tile([128, B], F16, tag="ws2", name="ws2")
            nc.scalar.activation(out=ws2[:], in_=ats2[:], func=AF.Exp)
            slw2 = se2.tile([128, B * 41], F16, tag="slw2", name="slw2")
            nc.vector.tensor_tensor(
                out=slw2[:].rearrange("p (j e) -> p j e", e=41),
                in0=h2l3[:, :, 0:41],
                in1=ws2[:].rearrange("p (j e) -> p j e", e=1)
                .to_broadcast([128, B, 41]),
                op=mult)
            lsl = lacc[:, b * B * 41:(b + 1) * B * 41]
            nc.scalar.copy(lsl, pse_t[:, 0:B * 41])
            nc.vector.tensor_tensor(out=lsl, in0=lsl, in1=slw2[:], op=aadd)
            off += T

        # ---- final log_softmax over all slots, one pass ----
        la3 = lacc[:].rearrange("p (j e) -> p j e", e=41)
        rc2 = sbe.tile([128, NSLOT], F32, tag="rc2", name="rc2")
        nc.vector.reciprocal(rc2[:].rearrange("p (j h) -> p j h", h=1),
                             la3[:, :, 40:41])
        lg = sbe.tile([128, NSLOT * 40], F32, tag="lg", name="lg")
        lg3 = lg[:].rearrange("p (j e) -> p j e", e=40)
        nc.vector.tensor_tensor(
            out=lg3, in0=la3[:, :, 0:40],
            in1=rc2[:].rearrange("p (j e) -> p j e", e=1)
            .to_broadcast([128, NSLOT, 40]), op=mult)
        nc.vector.tensor_tensor(
            out=lg3, in0=lg3,
            in1=b2t[:].rearrange("p (j e) -> p j e", j=1)
            .to_broadcast([128, NSLOT, 40]), op=aadd)
        ex = sbe.tile([128, NSLOT * 40], F16, tag="ex", name="ex")
        nc.scalar.activation(out=ex[:], in_=lg[:], func=AF.Exp)
        sm = sbe.tile([128, NSLOT], F32, tag="sm", name="sm")
        nc.vector.reduce_sum(sm[:].rearrange("p (j h) -> p j h", h=1),
                             ex[:].rearrange("p (j e) -> p j e", e=40),
                             axis=AX.X)
        ls = sbe.tile([128, NSLOT], F32, tag="ls", name="ls")
        nc.scalar.activation(out=ls[:], in_=sm[:], func=AF.Ln)
        nc.vector.tensor_tensor(
            out=lg3, in0=lg3,
            in1=ls[:].rearrange("p (j e) -> p j e", e=1)
            .to_broadcast([128, NSLOT, 40]), op=sub)
        nc.sync.dma_start(
            OUT[:].rearrange("(j p) e -> p j e", p=128),
            lg3)


_CACHE = {}


def kernel(x, edge_index, W1, att_src1, att_dst1, b1, W2, att_src2, att_dst2,
           b2):
    in_maps, meta = _prep(np.asarray(x), np.asarray(edge_index),
                          np.asarray(W1), np.asarray(att_src1),
                          np.asarray(att_dst1), np.asarray(W2),
                          np.asarray(att_src2), np.asarray(att_dst2),
                          b1=b1, b2=b2)
    key = meta["shapes"].tobytes()
    if key not in _CACHE:
        _CACHE[key] = _build(meta)
    nc = _CACHE[key]
    res = run_bass_kernel_spmd(nc, in_maps, core_ids=list(range(NCORES)))
    order_ch = meta["order_ch"]
    out = np.empty((N, NCLASS), np.float32)
    for k in range(NCORES):
        ok = np.asarray(res.results[k]["out"])
        for s in range(NSLOT):
            g = int(order_ch[s * NCORES + k])
            r0 = g * DCH
            if r0 >= N:
                continue
            nrow = min(DCH, N - r0)
            out[r0:r0 + nrow] = ok[s * DCH:s * DCH + nrow]
    return out



# revision 52
# speedup vs baseline: 1.9147x; 1.9147x over previous
#!/usr/bin/env python3
"""2-layer GAT on 8 NeuronCores (Bass/Tile) — v2 blocked design.

Sharding: global dst chunks (128 nodes) are size-sorted and dealt to
(slot, core) so the 8 chunks sharing a slot have near-equal edge counts
(SPMD static shapes pad to the max over cores).  Node features, the L1
table (tab1) and the L2 table (tab2) all live in the SAME assignment
layout (row = core*NPAD + slot*128 + dl), so one edge-index stream
serves both layers and source windows always line up.

Phases C/E process blocks of B=7 slots: one dma_gather per
(block, window), one indicator / attention / exp op per block, matmuls
accumulate per-slot psums packed 2-per-bank, epilogues batched per
block, log-softmax + OUT write once at the end.
"""
import sys
import numpy as np

sys.path.insert(0, "/opt/pypackages")
sys.path.insert(0, "/opt/trn_rl_repo")

import concourse.bass as bass
import concourse.bacc as bacc
import concourse.tile as tile
import concourse.mybir as mybir
from concourse.bass_utils import run_bass_kernel_spmd

# problem constants
N = 100000
F_IN = 512
NHID = 16
HEADS = 8
NCLASS = 40
NEG_SLOPE = 0.2

NCORES = 8
DCH = 128                    # dsts per chunk/slot
NSLOT = 98                   # slots per core
GCH = NSLOT * NCORES         # 784 global chunk slots (782 real + 2 pad)
NPAD = NSLOT * DCH           # 12544 rows per core (nodes, assignment order)
NSCH = 4
SCHW = 2 * NPAD              # 25088 table rows per window (cores {2w,2w+1})
B = 7                        # slots per block
NBLK = NSLOT // B            # 14 blocks

ROW1 = 128    # f16 elems per L1 table row (256B): h1 only (asrc separate)
G1W = 136     # f16 elems per L1 row slot in SBUF: [h1 128 | ws 8]
ROW2 = 128    # f16 elems per L2 table row (256B): [h2 40 | one | asrc2 | pad]
G2W = 42
ROWA = 128    # f16 elems per adst-table row (256B)

F16 = mybir.dt.float16
F32 = mybir.dt.float32
I16 = mybir.dt.int16


def _wrap(v):
    """Wrap an int16 stream (len % 16 == 0) into the dma_gather idx layout
    [16, L/16], replicated to 128 partitions."""
    w = v.reshape(-1, 16).T
    return np.tile(w, (8, 1))


def _dma_gather_raw(gp, out_ap, in_ap, idxs_ap, num_idxs, elem_size, elem_step,
                    queue_num=0):
    """dma_gather allowing elem_size (elems read per row) that is not a
    multiple of 256B; the table row stride (elem_step) still must be."""
    from concourse.bass import exact_div
    stride_bytes = elem_step * mybir.dt.size(in_ap.dtype)
    stride_bytes_256 = exact_div(stride_bytes, 256)
    _in_ap = gp.lower_ap_dma(in_ap, for_custom_bir_dma=True)
    _idxs_ap = gp.lower_ap(idxs_ap)
    _out_ap = gp.lower_ap(out_ap)
    return gp.add_instruction(
        mybir.InstDMAGatherAnt(
            name=gp.bass.get_next_instruction_name(),
            ins=[*_in_ap, _idxs_ap, gp.lower_val_access(gp.to_reg(num_idxs))],
            outs=[_out_ap],
            transpose=False, num_idxs=num_idxs, elem_size=elem_size,
            stride_bytes_256=stride_bytes_256, gen_mode=0,
            single_packet=False, queue_num=queue_num,
            sbuf_tokens_per_rank=0, sbuf_free_dim_per_rank=0,
            sbuf_free_dim_pad_per_rank=0, sbuf_byte_offset=0))


def _prep(x, edge_index, W1, att_src1, att_dst1, W2, att_src2, att_dst2,
          b1=None, b2=None):
    """Host-side sharding/packing. Returns (in_maps, meta)."""
    # self-loops are handled analytically in the epilogues (no gather), so
    # the edge stream carries only the real edges
    x = np.asarray(x, np.float32)
    src = np.asarray(edge_index[0]).astype(np.int64)
    dst = np.asarray(edge_index[1]).astype(np.int64)

    # --- sorted chunk -> (slot, core) assignment ---------------------------
    gch_d = dst // DCH                            # 0..781
    sizes = np.bincount(gch_d, minlength=GCH)
    order_ch = np.argsort(-sizes, kind="stable")  # descending size
    asn_core = np.empty(GCH, np.int64)
    asn_slot = np.empty(GCH, np.int64)
    rr = np.arange(GCH)
    asn_slot[order_ch] = rr // NCORES
    asn_core[order_ch] = rr % NCORES

    core_e = asn_core[gch_d]                      # dst-owner core per edge
    slot_e = asn_slot[gch_d]
    dl_e = dst % DCH

    # --- source position in the (shared) assignment-layout table -----------
    gch_s = src // DCH
    sc = asn_core[gch_s]
    sp = sc * NPAD + asn_slot[gch_s] * DCH + (src % DCH)
    sch = sp // SCHW                              # window = src core pair
    sloc = sp - sch * SCHW                        # 0..25087, int16-safe

    # --- cells: (dst core, block, window, slot-in-block), pad each to 128 --
    blk = slot_e // B
    sib = slot_e % B
    cell = ((core_e * NBLK + blk) * NSCH + sch) * B + sib
    npc_cells = NBLK * NSCH * B                   # cells per core
    order = np.argsort(cell * (SCHW + 1) + sloc, kind="stable")
    cell_s = cell[order]
    counts = np.bincount(cell_s, minlength=NCORES * npc_cells)
    shapes = (np.ceil(counts.reshape(NCORES, npc_cells).max(axis=0)
                      / 16.0) * 16).astype(np.int64)        # [npc_cells]
    # pad each (blk, window) run of B cells to a multiple of 128 by
    # inflating the run's last cell
    shp4 = shapes.reshape(NBLK, NSCH, B)
    shp4[:, :, B - 1] += (-shp4.sum(axis=2)) % 128
    shapes = shp4.reshape(-1)
    cell_starts = np.concatenate([[0], np.cumsum(shapes)])
    L = int(cell_starts[-1])
    t_total = L // 128

    # slab table: one (tile, cell) pair per tile a cell overlaps.  Shared
    # across cores (derived from shapes alone).  Slabs sorted by (tile, cell).
    pairs = []
    for c in range(npc_cells):
        cs, ce = int(cell_starts[c]), int(cell_starts[c + 1])
        for t in range(cs // 128, (ce + 127) // 128):
            pairs.append((t, c))
    pairs.sort()
    slab_of = {}
    slab_tile = np.empty(len(pairs), np.int64)
    slab_cell = np.empty(len(pairs), np.int64)
    for i, (t, c) in enumerate(pairs):
        slab_of[(c, t)] = i
        slab_tile[i] = t
        slab_cell[i] = c
    nslab = len(pairs)

    group_start = np.concatenate([[0], np.cumsum(counts)])
    rank = np.arange(len(cell_s)) - group_start[cell_s]
    pos = cell_starts[cell_s % npc_cells] + rank
    core_s = cell_s // npc_cells

    IDX1 = np.zeros((NCORES, L), np.int16)
    IDXD = np.zeros((NCORES, L), np.int16)
    IDX1[core_s, pos] = sloc[order].astype(np.int16)
    IDXD[core_s, pos] = (slot_e * DCH + dl_e)[order].astype(np.int16)

    # dstloc stream indexed by SLAB: edges of (cell c, tile t) land in slab
    # slab_of[(c, t)]; other slabs covering tile t keep 255 at that partition
    cell_local_e = cell_s % npc_cells
    tile_e = pos // 128
    slab_e = np.array([slab_of[(int(c), int(t))]
                       for c, t in zip(cell_local_e, tile_e)], np.int64)
    DSTL = np.full((NCORES, nslab * 128), 255.0, np.float32)
    DSTL[core_s, slab_e * 128 + pos % 128] = dl_e[order].astype(np.float32)

    IDX1w = np.stack([_wrap(IDX1[k]) for k in range(NCORES)])
    IDXDw = np.stack([_wrap(IDXD[k]) for k in range(NCORES)])
    DSTLw = DSTL.reshape(NCORES, nslab, 128).transpose(0, 2, 1).copy()

    # --- weights -----------------------------------------------------------
    asrc1 = np.asarray(att_src1).reshape(HEADS, NHID)
    adst1 = np.asarray(att_dst1).reshape(HEADS, NHID)
    W1r = np.asarray(W1).reshape(F_IN, HEADS, NHID)
    W1as = np.einsum("khc,hc->kh", W1r, asrc1)
    W1ad = np.einsum("khc,hc->kh", W1r, adst1)
    W1ext = np.concatenate([np.asarray(W1), W1as, W1ad],
                           axis=1).astype(np.float16)          # [512, 144]
    W2as = np.asarray(W2) @ np.asarray(att_src2).reshape(NCLASS, 1)
    W2ad = np.asarray(W2) @ np.asarray(att_dst2).reshape(NCLASS, 1)
    W2ext = np.concatenate([np.asarray(W2), W2as, W2ad],
                           axis=1).astype(np.float16)          # [128, 42]

    iota = np.broadcast_to(np.arange(128, dtype=np.float16), (128, 128)).copy()

    # --- per-core node features in assignment order ------------------------
    xpad = np.zeros((GCH * DCH, F_IN), np.float16)
    xpad[:N] = x.astype(np.float16)
    in_maps = []
    for k in range(NCORES):
        chunks = order_ch[np.arange(NSLOT) * NCORES + k]       # slot -> chunk
        xs = xpad.reshape(GCH, DCH, F_IN)[chunks]              # [98,128,512]
        xT = np.ascontiguousarray(
            xs.reshape(NPAD, F_IN).T)                          # [512, 12544]
        in_maps.append({
            "xT": xT, "W1ext": W1ext, "W2ext": W2ext,
            "IDX1": IDX1w[k], "IDXD": IDXDw[k], "DSTLOC": DSTLw[k],
            "iota": iota,
            "B1": (np.zeros((1, 128), np.float32) if b1 is None
                   else np.asarray(b1, np.float32).reshape(1, 128)),
            "B2": (np.zeros((1, NCLASS), np.float32) if b2 is None
                   else np.asarray(b2, np.float32).reshape(1, NCLASS)),
        })
    meta = {"shapes": shapes.reshape(NBLK, NSCH, B),
            "t_total": t_total, "order_ch": order_ch,
            "nslab": nslab, "slab_tile": slab_tile, "slab_cell": slab_cell,
            "cell_starts": cell_starts}
    return in_maps, meta


def _build(meta, phases="ABCDE", clevel=9):
    from concourse.masks import make_identity

    shp = meta["shapes"]            # [NBLK, NSCH, B] padded edge counts
    t_total = meta["t_total"]
    blk_tiles = [int(shp[b].sum()) // 128 for b in range(NBLK)]
    TMAX = max(blk_tiles)
    # slab structure: per block, list of (slot j, slab-local idx, tile-local)
    nslab = int(meta["nslab"])
    slab_tile = meta["slab_tile"]
    slab_cell = meta["slab_cell"]
    blk_of = slab_cell // (NSCH * B)
    slab_off = [int(np.searchsorted(blk_of, b)) for b in range(NBLK + 1)]
    tile_off = np.concatenate([[0], np.cumsum(blk_tiles)])
    blk_mm = []
    for b in range(NBLK):
        lo, hi = slab_off[b], slab_off[b + 1]
        blk_mm.append([(int(slab_cell[i]) % B, i - lo,
                        int(slab_tile[i]) - int(tile_off[b]))
                       for i in range(lo, hi)])
    blk_ns = [slab_off[b + 1] - slab_off[b] for b in range(NBLK)]
    NSMAX = max(blk_ns)
    meta_rt = (blk_mm, blk_ns, slab_off, NSMAX)

    nc = bacc.Bacc("TRN2", target_bir_lowering=False, debug=False,
                   enable_asserts=False, num_devices=NCORES,
                   num_swdge_queues=4)

    xT = nc.dram_tensor("xT", [F_IN, NPAD], F16, kind="ExternalInput")
    W1e = nc.dram_tensor("W1ext", [F_IN, 144], F16, kind="ExternalInput")
    W2e = nc.dram_tensor("W2ext", [128, 42], F16, kind="ExternalInput")
    IDX1 = nc.dram_tensor("IDX1", [128, t_total * 8], I16, kind="ExternalInput")
    IDXD = nc.dram_tensor("IDXD", [128, t_total * 8], I16, kind="ExternalInput")
    DSTLOC = nc.dram_tensor("DSTLOC", [128, nslab], F32, kind="ExternalInput")
    IOTA = nc.dram_tensor("iota", [128, 128], F16, kind="ExternalInput")
    B1 = nc.dram_tensor("B1", [1, 128], F32, kind="ExternalInput")
    B2 = nc.dram_tensor("B2", [1, NCLASS], F32, kind="ExternalInput")
    OUT = nc.dram_tensor("out", [NPAD, NCLASS], F32, kind="ExternalOutput")

    tab1_sh = nc.dram_tensor("tab1_sh", [NPAD, ROW1], F16, kind="Internal")
    # one spare row: the g1 gather reads 136 elems from 128-elem-stride rows
    # (16B overread into the following row; cols 128:136 are overwritten by
    # the ws copy), so the last row must not fall off the allocation
    tab1 = nc.dram_tensor("tab1", [NPAD * NCORES + 1, ROW1], F16,
                          kind="Internal", addr_space="Shared")
    asrc_sh = nc.dram_tensor("asrc_sh", [NPAD, 8], F16, kind="Internal")
    asrc_all = nc.dram_tensor("asrc_all", [NPAD * NCORES, 8], F16,
                              kind="Internal", addr_space="Shared")
    asrc_tab = nc.dram_tensor("asrc_tab", [NPAD * NCORES, 128], F16,
                              kind="Internal")
    tab2_sh = nc.dram_tensor("tab2_sh", [NPAD, 42], F16, kind="Internal")
    tab2_all = nc.dram_tensor("tab2_all", [NPAD * NCORES, 42], F16,
                              kind="Internal", addr_space="Shared")
    tab2 = nc.dram_tensor("tab2", [NPAD * NCORES, ROW2], F16, kind="Internal")
    adr1 = nc.dram_tensor("adr1", [NPAD, ROWA], F16, kind="Internal")
    adr2 = nc.dram_tensor("adr2", [NPAD, ROWA], F16, kind="Internal")

    with tile.TileContext(nc) as tc:
        if "A" in phases:
            _phase_a(nc, tc, xT, W1e, tab1_sh, adr1, asrc_sh)
        if "B" in phases:
            # small dense asrc collective first: its consumers (restride +
            # per-edge asrc gathers + attention weights) then overlap the
            # big tab1 collective
            nc.gpsimd.collective_compute(
                "AllGather", mybir.AluOpType.bypass,
                replica_groups=[list(range(NCORES))],
                ins=[asrc_sh[:]], outs=[asrc_all[:]])
            for s in range(NSCH):
                nc.sync.dma_start(
                    asrc_tab[s * SCHW:(s + 1) * SCHW, 0:8],
                    asrc_all[s * SCHW:(s + 1) * SCHW, :])
            nc.gpsimd.collective_compute(
                "AllGather", mybir.AluOpType.bypass,
                replica_groups=[list(range(NCORES))],
                ins=[tab1_sh[:]], outs=[tab1[0:NPAD * NCORES, :]])
        with tc.tile_pool(name="sbSH", bufs=1) as shp_pool:
            shared = {
                "slw2": shp_pool.tile([128, NSLOT * 41], F16, tag="slw2a",
                                      name="slw2a"),
                "ga2all": shp_pool.tile([128, t_total * 8], F16, tag="ga2all",
                                        name="ga2all"),
            }
            if "C" in phases:
                _phase_c(nc, tc, shp, blk_tiles, TMAX, make_identity,
                         IDX1, IDXD, DSTLOC, IOTA, B1, W2e, tab1, tab1_sh,
                         adr1, asrc_sh, asrc_tab, tab2_sh, adr2, clevel,
                         meta_rt, shared)
            if "D" in phases:
                # dense AllGather (8.4MB instead of 25.7MB of padded rows),
                # then per-window restride into the 256B-row gather table so
                # each window's gathers start as soon as its chunk lands
                nc.gpsimd.collective_compute(
                    "AllGather", mybir.AluOpType.bypass,
                    replica_groups=[list(range(NCORES))],
                    ins=[tab2_sh[:]], outs=[tab2_all[:]])
                for s in range(NSCH):
                    nc.sync.dma_start(
                        tab2[s * SCHW:(s + 1) * SCHW, 0:42],
                        tab2_all[s * SCHW:(s + 1) * SCHW, :])
            if "E" in phases:
                _phase_e(nc, tc, shp, blk_tiles, TMAX,
                         IDX1, IDXD, DSTLOC, IOTA, B2, tab2, tab2_sh, adr2,
                         OUT, clevel, meta_rt, shared)
            else:
                with tc.tile_pool(name="sbZ", bufs=1) as sbz:
                    z = sbz.tile([128, NSLOT * 40], F32, tag="z", name="z")
                    nc.vector.memset(z[:], 0.0)
                    nc.sync.dma_start(
                        OUT[:].rearrange("(j p) e -> p j e", p=128),
                        z[:].rearrange("p (j e) -> p j e", e=40))

    nc.compile()
    return nc


def _phase_a(nc, tc, xT, W1e, tab1_sh, adr1, asrc_sh):
    """h1 = x @ W1ext per 128-node slot; writes tab1_sh (h1), asrc_sh
    (dense asrc) and adr1 (adst), batched 7 slots per DMA.

    The attention columns (asrc/adst, cols 128:144 of W1ext) are computed
    for ALL slots first so the small asrc/adst collective and its consumers
    (the per-edge attention-weight pre-pass) start ~80us before the big h1
    table is even finished."""
    AG = 7
    with tc.tile_pool(name="sbA", bufs=1) as sba, \
         tc.tile_pool(name="sbA2", bufs=3) as sba2, \
         tc.tile_pool(name="psA", bufs=4, space="PSUM") as psa:
        xts = [sba.tile([128, NPAD], F16, tag=f"xt{k}", name=f"xt{k}")
               for k in range(4)]
        w1s = [sba.tile([128, 144], F16, tag=f"w1{k}", name=f"w1{k}")
               for k in range(4)]
        for k in range(4):
            nc.sync.dma_start(xts[k][:], xT[k * 128:(k + 1) * 128, :])
            nc.sync.dma_start(w1s[k][:], W1e[k * 128:(k + 1) * 128, :])
        # pass 1: attention columns only (asrc | adst), all slots; AG slots
        # share one psum bank so extraction is two strided copies per group
        for gb in range(NSLOT // AG):
            s8 = sba2.tile([128, AG * 8], F16, tag="s8A", name="s8A")
            t8 = sba2.tile([128, AG * 8], F16, tag="t8A", name="t8A")
            ps = psa.tile([128, AG * 16], F32, tag="psAa", name="psAa")
            for j in range(AG):
                nt = gb * AG + j
                for k in range(4):
                    nc.tensor.matmul(ps[:, j * 16:(j + 1) * 16],
                                     lhsT=xts[k][:, nt * 128:(nt + 1) * 128],
                                     rhs=w1s[k][:, 128:144], start=(k == 0),
                                     stop=(k == 3))
            ps3 = ps[:].rearrange("p (j e) -> p j e", e=16)
            nc.vector.tensor_copy(
                s8[:].rearrange("p (j e) -> p j e", e=8), ps3[:, :, 0:8])
            nc.vector.tensor_copy(
                t8[:].rearrange("p (j e) -> p j e", e=8), ps3[:, :, 8:16])
            nc.sync.dma_start(
                asrc_sh[gb * AG * 128:(gb + 1) * AG * 128, 0:8]
                .rearrange("(g p) e -> p g e", p=128),
                s8[:].rearrange("p (g e) -> p g e", e=8))
            nc.sync.dma_start(
                adr1[gb * AG * 128:(gb + 1) * AG * 128, 0:8]
                .rearrange("(g p) e -> p g e", p=128),
                t8[:].rearrange("p (g e) -> p g e", e=8))
        # pass 2: h1 table
        for gb in range(NSLOT // AG):
            row = sba2.tile([128, AG * 128], F16, tag="rowA", name="rowA")
            for j in range(AG):
                nt = gb * AG + j
                ps = psa.tile([128, 128], F32, tag="psA", name="psA")
                for k in range(4):
                    nc.tensor.matmul(ps[:],
                                     lhsT=xts[k][:, nt * 128:(nt + 1) * 128],
                                     rhs=w1s[k][:, 0:128], start=(k == 0),
                                     stop=(k == 3))
                nc.vector.tensor_copy(row[:, j * 128:(j + 1) * 128],
                                      ps[:, 0:128])
            nc.sync.dma_start(
                tab1_sh[gb * AG * 128:(gb + 1) * AG * 128, 0:128]
                .rearrange("(g p) e -> p g e", p=128),
                row[:].rearrange("p (g e) -> p g e", e=128))


def _phase_c(nc, tc, shp, blk_tiles, TMAX, make_identity,
             IDX1, IDXD, DSTLOC, IOTA, B1, W2e, tab1, tab1_sh, adr1,
             asrc_sh, asrc_tab, tab2_sh, adr2, clevel=9, meta_rt=None,
             shared=None):
    blk_mm, blk_ns, slab_off, NSMAX = meta_rt
    eq = mybir.AluOpType.is_equal
    mult = mybir.AluOpType.mult
    amax = mybir.AluOpType.max
    aadd = mybir.AluOpType.add
    AF = mybir.ActivationFunctionType

    with tc.tile_pool(name="sbC", bufs=1) as sbc, \
         tc.tile_pool(name="psC", bufs=1, space="PSUM") as psc, \
         tc.tile_pool(name="psC2", bufs=2, space="PSUM") as psc2:
        iot = sbc.tile([128, 128], F16, tag="iota", name="iotc")
        nc.sync.dma_start(iot[:], IOTA[:])
        ident = sbc.tile([128, 128], F16, tag="ident", name="ident")
        make_identity(nc, ident[:])
        w2s = sbc.tile([128, 42], F16, tag="w2s", name="w2s")
        nc.sync.dma_start(w2s[:], W2e[:])
        b1t = sbc.tile([128, 128], F32, tag="b1t", name="b1t")
        nc.sync.dma_start(b1t[:], B1[:].to_broadcast([128, 128]))
        t_tot = sum(blk_tiles)
        wsa = sbc.tile([128, t_tot * 8], F16, tag="wsa", name="wsa")

        # ---- ws pre-pass: per-edge attention weights for the whole layer.
        # Depends only on adr1 / asrc_tab (local + small collective), so it
        # all runs during the big tab1 AllGather.
        with tc.tile_pool(name="sbCP2", bufs=3) as sbp2:
            off = 0
            for b in range(NBLK):
                T = blk_tiles[b]
                i1p = sbp2.tile([128, TMAX * 8], I16, tag="i1p", name="i1p")
                nc.sync.dma_start(i1p[:, 0:T * 8],
                                  IDX1[:, off * 8:(off + T) * 8])
                iddp = sbp2.tile([128, TMAX * 8], I16, tag="iddp", name="iddp")
                nc.sync.dma_start(iddp[:, 0:T * 8],
                                  IDXD[:, off * 8:(off + T) * 8])
                gsr = sbp2.tile([128, TMAX * 8], F16, tag="gsr", name="gsr")
                coff = 0
                for s in range(NSCH):
                    cl = int(shp[b, s].sum())
                    if cl == 0:
                        continue
                    for p0 in range(coff, coff + cl, 8192):
                        pl = min(8192, coff + cl - p0)
                        _dma_gather_raw(
                            nc.gpsimd,
                            gsr[:, (p0 // 128) * 8:((p0 + pl) // 128) * 8]
                            .rearrange("p (t e) -> p t e", e=8),
                            asrc_tab[s * SCHW:(s + 1) * SCHW, :],
                            i1p[:, p0 // 16:(p0 + pl) // 16], pl, 8, 128,
                            queue_num=s)
                    coff += cl
                nedge = T * 128
                gad = sbp2.tile([128, TMAX * 8], F16, tag="gad", name="gad")
                for p0 in range(0, nedge, 8192):
                    pl = min(8192, nedge - p0)
                    _dma_gather_raw(
                        nc.gpsimd,
                        gad[:, (p0 // 128) * 8:((p0 + pl) // 128) * 8]
                        .rearrange("p (t e) -> p t e", e=8),
                        adr1[:], iddp[:, p0 // 16:(p0 + pl) // 16], pl, 8,
                        ROWA, queue_num=(b + p0 // 8192) % 4)
                att = sbp2.tile([128, TMAX * 8], F16, tag="attp", name="attp")
                nc.vector.tensor_tensor(out=att[:, 0:T * 8],
                                        in0=gsr[:, 0:T * 8],
                                        in1=gad[:, 0:T * 8], op=aadd)
                nc.vector.scalar_tensor_tensor(
                    out=att[:, 0:T * 8], in0=att[:, 0:T * 8],
                    scalar=NEG_SLOPE, in1=att[:, 0:T * 8], op0=mult, op1=amax)
                nc.scalar.activation(out=wsa[:, off * 8:(off + T) * 8],
                                     in_=att[:, 0:T * 8], func=AF.Exp)
                off += T

        _phase_c_main(nc, tc, shp, blk_tiles, TMAX, iot, ident, w2s, b1t,
                      wsa, IDX1, IDXD, DSTLOC, tab1, tab1_sh, adr1, asrc_sh,
                      tab2_sh, adr2, clevel, meta_rt, shared, psc, psc2)


def _phase_c_main(nc, tc, shp, blk_tiles, TMAX, iot, ident, w2s, b1t, wsa,
                  IDX1, IDXD, DSTLOC, tab1, tab1_sh, adr1, asrc_sh, tab2_sh,
                  adr2, clevel, meta_rt, shared, psc, psc2):
    blk_mm, blk_ns, slab_off, NSMAX = meta_rt
    eq = mybir.AluOpType.is_equal
    mult = mybir.AluOpType.mult
    amax = mybir.AluOpType.max
    aadd = mybir.AluOpType.add
    AF = mybir.ActivationFunctionType

    with tc.tile_pool(name="sbC2", bufs=2) as sb2, \
         tc.tile_pool(name="sbC3", bufs=2) as sb3, \
         tc.tile_pool(name="sbCW", bufs=1) as sbw:
        off = 0
        for b in range(NBLK):
            T = blk_tiles[b]
            i1 = sb2.tile([128, TMAX * 8], I16, tag="i1", name="i1")
            nc.sync.dma_start(i1[:, 0:T * 8], IDX1[:, off * 8:(off + T) * 8])
            NS = blk_ns[b]
            dlc = sb2.tile([128, NSMAX], F32, tag="dlc", name="dlc")
            nc.sync.dma_start(dlc[:, 0:NS],
                              DSTLOC[:, slab_off[b]:slab_off[b] + NS])

            g1 = sb2.tile([128, TMAX * G1W], F16, tag="g1", name="g1")
            coff = 0
            for s in range(NSCH):
                cl = int(shp[b, s].sum())
                if cl == 0:
                    continue
                for p0 in range(coff, coff + cl, 8192):
                    pl = min(8192, coff + cl - p0)
                    _dma_gather_raw(
                        nc.gpsimd,
                        g1[:, (p0 // 128) * G1W:((p0 + pl) // 128) * G1W]
                        .rearrange("p (t e) -> p t e", e=G1W),
                        tab1[s * SCHW:s * SCHW + SCHW + 1, :],
                        i1[:, p0 // 16:(p0 + pl) // 16], pl, G1W, ROW1,
                        queue_num=s)
                coff += cl

            if clevel < 2:
                off += T
                continue
            g13 = g1[:, 0:T * G1W].rearrange("p (t e) -> p t e", e=G1W)

            # ws (precomputed exp weights) into the g1 row slots: becomes the
            # denominator columns of the segment matmul
            nc.scalar.copy(g13[:, :, 128:136],
                           wsa[:, off * 8:(off + T) * 8]
                           .rearrange("p (t h) -> p t h", h=8))
            # expanded weights (one per channel) on the Activation engine so
            # the big feature multiply stays packed (2 elem/cycle on DVE).
            # wexp aliases the front of the ind tile: it is dead once the
            # multiply finishes, before any ind slab is built.
            ind = sbw.tile([128, NSMAX * 128], F16, tag="ind", name="ind")
            wexp = ind[:, 0:T * 128]
            nc.scalar.copy(
                wexp.rearrange("p (t h c) -> p t h c", h=8, c=NHID),
                wsa[:, off * 8:(off + T) * 8]
                .rearrange("p (t h c) -> p t h c", h=8, c=1)
                .to_broadcast([128, T, 8, NHID]))
            nc.vector.tensor_tensor(
                out=g13[:, :, 0:128],
                in0=g13[:, :, 0:128],
                in1=wexp.rearrange("p (t e) -> p t e", e=128),
                op=mult)

            if clevel < 3:
                off += T
                continue
            # indicator one-hot per (edge, slab): per-slab tensor_scalar with
            # the slab's dst-local column as a per-partition f32 scalar hits
            # the DVE 2x/4x packed fast path
            for sl in range(NS):
                nc.vector.tensor_scalar(
                    out=ind[:, sl * 128:(sl + 1) * 128], in0=iot[:],
                    scalar1=dlc[:, sl:sl + 1], scalar2=None, op0=eq)

            # segment sums into per-slot psums (2 slots per bank)
            pss = [psc.tile([128, 272], F32, tag=f"pss{j}", name=f"pss{j}")
                   for j in range((B + 1) // 2)]
            for ps in pss:
                nc.vector.memset(ps[:], 0.0)
            mm = blk_mm[b]
            last_of = {}
            for i, (j, sl, tl) in enumerate(mm):
                last_of[j] = i
            for i, (j, sl, tl) in enumerate(mm):
                ps = pss[j // 2]
                nc.tensor.matmul(
                    ps[:, (j % 2) * 136:(j % 2) * 136 + 136],
                    lhsT=ind[:, sl * 128:(sl + 1) * 128],
                    rhs=g1[:, tl * G1W:tl * G1W + 136],
                    start=False, stop=(last_of[j] == i))

            if clevel < 4:
                off += T
                continue
            # ---- epilogue, batched over the block ----
            # self-loop contribution: w_self * h1[d] added locally (no gather)
            h1l = sb3.tile([128, B * 128], F16, tag="h1l", name="h1l")
            nc.sync.dma_start(
                h1l[:].rearrange("p (j e) -> p j e", e=128),
                tab1_sh[b * B * 128:(b + 1) * B * 128, 0:128]
                .rearrange("(j p) e -> p j e", p=128))
            sa = sb3.tile([128, B * 8], F16, tag="sa", name="sa")
            nc.sync.dma_start(
                sa[:].rearrange("p (j e) -> p j e", e=8),
                asrc_sh[b * B * 128:(b + 1) * B * 128, 0:8]
                .rearrange("(j p) e -> p j e", p=128))
            aa = sb3.tile([128, B * 8], F16, tag="aa", name="aa")
            nc.sync.dma_start(
                aa[:].rearrange("p (j e) -> p j e", e=8),
                adr1[b * B * 128:(b + 1) * B * 128, 0:8]
                .rearrange("(j p) e -> p j e", p=128))
            h1l3 = h1l[:].rearrange("p (j e) -> p j e", e=128)
            ats = sb3.tile([128, B * 8], F16, tag="ats", name="ats")
            ats3 = ats[:].rearrange("p (j h) -> p j h", h=8)
            nc.vector.tensor_tensor(out=ats3,
                                    in0=sa[:].rearrange("p (j h) -> p j h", h=8),
                                    in1=aa[:].rearrange("p (j h) -> p j h", h=8),
                                    op=aadd)
            nc.vector.scalar_tensor_tensor(
                out=ats3, in0=ats3, scalar=NEG_SLOPE, in1=ats3,
                op0=mult, op1=amax)
            ws = sb3.tile([128, B * 8], F16, tag="ws", name="ws")
            nc.scalar.activation(out=ws[:], in_=ats[:], func=AF.Exp)
            slw = sb3.tile([128, B * 128], F16, tag="slw", name="slw")
            nc.vector.tensor_tensor(
                out=slw[:].rearrange("p (j h c) -> p j h c", h=8, c=NHID),
                in0=h1l3[:, :, 0:128].rearrange("p j (h c) -> p j h c", c=NHID),
                in1=ws[:].rearrange("p (j h c) -> p j h c", h=8, c=1)
                .to_broadcast([128, B, 8, NHID]),
                op=mult)

            ob = sbw.tile([128, B * 136], F32, tag="ob", name="ob")
            for j in range(B):
                nc.scalar.copy(ob[:, j * 136:(j + 1) * 136],
                               pss[j // 2][:, (j % 2) * 136:(j % 2) * 136 + 136])
            ob3 = ob[:].rearrange("p (j e) -> p j e", e=136)
            nc.vector.tensor_tensor(
                out=ob3[:, :, 0:128], in0=ob3[:, :, 0:128],
                in1=slw[:].rearrange("p (j e) -> p j e", e=128), op=aadd)
            nc.vector.tensor_tensor(
                out=ob3[:, :, 128:136], in0=ob3[:, :, 128:136],
                in1=ws[:].rearrange("p (j h) -> p j h", h=8), op=aadd)
            rc = sb3.tile([128, B * 8], F32, tag="rc", name="rc")
            nc.vector.reciprocal(rc[:].rearrange("p (j h) -> p j h", h=8),
                                 ob3[:, :, 128:136])
            o1 = sb3.tile([128, B * 128], F32, tag="o1", name="o1")
            nc.vector.tensor_tensor(
                out=o1[:].rearrange("p (j h c) -> p j h c", h=8, c=NHID),
                in0=ob3[:, :, 0:128].rearrange("p j (h c) -> p j h c", c=NHID),
                in1=rc[:].rearrange("p (j h c) -> p j h c", h=8, c=1)
                .to_broadcast([128, B, 8, NHID]),
                op=mult)
            nc.vector.tensor_tensor(
                out=o1[:].rearrange("p (j e) -> p j e", e=128),
                in0=o1[:].rearrange("p (j e) -> p j e", e=128),
                in1=b1t[:].rearrange("p (j e) -> p j e", j=1)
                .to_broadcast([128, B, 128]), op=aadd)
            # elu = max(x,0) + exp(min(x,0)) - 1   (fp16 exp path)
            t1 = sb3.tile([128, B * 128], F16, tag="t1", name="t1")
            nc.gpsimd.tensor_scalar_min(t1[:], o1[:], 0.0)
            t2 = sb3.tile([128, B * 128], F16, tag="t2", name="t2")
            nc.scalar.activation(out=t2[:], in_=t1[:], func=AF.Exp)
            nc.gpsimd.tensor_scalar_add(t2[:], t2[:], -1.0)
            nc.vector.tensor_scalar_max(o1[:], o1[:], 0.0)
            elu = sb3.tile([128, B * 128], F16, tag="elu", name="elu")
            nc.vector.tensor_tensor(out=elu[:], in0=o1[:], in1=t2[:], op=aadd)

            if clevel < 5:
                off += T
                continue
            # layer-2 rows: h2 = eluT @ W2ext per slot
            h2 = sb3.tile([128, B * 42], F16, tag="h2", name="h2")
            a2 = sb3.tile([128, B * 8], F16, tag="a2", name="a2")
            for j in range(B):
                psT = psc2.tile([128, 128], F16, tag="psT", name="psT")
                nc.tensor.transpose(psT[:], elu[:, j * 128:(j + 1) * 128],
                                    ident[:])
                eluT = sb3.tile([128, 128], F16, tag="eluT", name="eluT")
                nc.scalar.copy(eluT[:], psT[:])
                ps2a = psc2.tile([128, 42], F32, tag="ps2a", name="ps2a")
                nc.tensor.matmul(ps2a[:], lhsT=eluT[:], rhs=w2s[:],
                                 start=True, stop=True)
                nc.vector.tensor_copy(h2[:, j * 42:j * 42 + NCLASS],
                                      ps2a[:, 0:NCLASS])
                nc.vector.memset(h2[:, j * 42 + NCLASS:j * 42 + NCLASS + 1],
                                 1.0)
                nc.vector.tensor_copy(h2[:, j * 42 + 41:j * 42 + 42],
                                      ps2a[:, NCLASS:NCLASS + 1])
                nc.vector.tensor_copy(
                    a2[:, j * 8:(j + 1) * 8].rearrange("p (r h) -> p r h", h=1),
                    ps2a[:, 41:42].rearrange("p (r h) -> p r h", r=1)
                    .to_broadcast([128, 8, 1]))
            nc.sync.dma_start(
                tab2_sh[b * B * 128:(b + 1) * B * 128, 0:42]
                .rearrange("(j p) e -> p j e", p=128),
                h2[:].rearrange("p (j e) -> p j e", e=42))
            nc.sync.dma_start(
                adr2[b * B * 128:(b + 1) * B * 128, 0:8]
                .rearrange("(j p) e -> p j e", p=128),
                a2[:].rearrange("p (j e) -> p j e", e=8))
            # layer-2 self-loop contribution, precomputed here (h2/a2 are
            # in SBUF) so phase E needs no local loads for it
            h23 = h2[:].rearrange("p (j e) -> p j e", e=42)
            ats2 = sb3.tile([128, B], F16, tag="ats2c", name="ats2c")
            a23 = ats2[:].rearrange("p (j h) -> p j h", h=1)
            nc.vector.tensor_tensor(
                out=a23, in0=h23[:, :, 41:42],
                in1=a2[:].rearrange("p (j h) -> p j h", h=8)[:, :, 0:1],
                op=aadd)
            nc.vector.scalar_tensor_tensor(
                out=a23, in0=a23, scalar=NEG_SLOPE, in1=a23,
                op0=mult, op1=amax)
            ws2 = sb3.tile([128, B], F16, tag="ws2c", name="ws2c")
            nc.scalar.activation(out=ws2[:], in_=ats2[:], func=AF.Exp)
            nc.vector.tensor_tensor(
                out=shared["slw2"][:, b * B * 41:(b + 1) * B * 41]
                .rearrange("p (j e) -> p j e", e=41),
                in0=h23[:, :, 0:41],
                in1=ws2[:].rearrange("p (j e) -> p j e", e=1)
                .to_broadcast([128, B, 41]),
                op=mult)
            # adst2 gather for this block (overlaps later C blocks and the
            # tab2 AllGather; adr2 rows for this block were just written)
            idd2 = sbw.tile([128, TMAX * 8], I16, tag="idd2", name="idd2")
            nc.sync.dma_start(idd2[:, 0:T * 8],
                              IDXD[:, off * 8:(off + T) * 8])
            for p0 in range(0, T * 128, 8192):
                pl = min(8192, T * 128 - p0)
                _dma_gather_raw(
                    nc.gpsimd,
                    shared["ga2all"][:, (off + p0 // 128) * 8:
                                     (off + (p0 + pl) // 128) * 8]
                    .rearrange("p (t e) -> p t e", e=8),
                    adr2[0:(b + 1) * B * 128, :],
                    idd2[:, p0 // 16:(p0 + pl) // 16],
                    pl, 8, ROWA, queue_num=(b + p0 // 8192) % 4)
            off += T


def _phase_e(nc, tc, shp, blk_tiles, TMAX,
             IDX1, IDXD, DSTLOC, IOTA, B2, tab2, tab2_sh, adr2, OUT,
             clevel=9, meta_rt=None, shared=None):
    blk_mm, blk_ns, slab_off, NSMAX = meta_rt
    eq = mybir.AluOpType.is_equal
    mult = mybir.AluOpType.mult
    amax = mybir.AluOpType.max
    aadd = mybir.AluOpType.add
    sub = mybir.AluOpType.subtract
    AF = mybir.ActivationFunctionType
    AX = mybir.AxisListType

    with tc.tile_pool(name="sbE", bufs=1) as sbe, \
         tc.tile_pool(name="sbE2", bufs=2) as se2, \
         tc.tile_pool(name="psE", bufs=2, space="PSUM") as pse:
        iot = sbe.tile([128, 128], F16, tag="iotaE", name="iote")
        nc.sync.dma_start(iot[:], IOTA[:])
        b2t = sbe.tile([128, NCLASS], F32, tag="b2t", name="b2t")
        nc.sync.dma_start(b2t[:], B2[:].to_broadcast([128, NCLASS]))
        lacc = sbe.tile([128, NSLOT * 41], F32, tag="lacc", name="lacc")

        ga2a = shared["ga2all"]

        off = 0
        for b in range(NBLK):
            T = blk_tiles[b]
            NS = blk_ns[b]
            i1 = se2.tile([128, TMAX * 8], I16, tag="i1e", name="i1e")
            nc.sync.dma_start(i1[:, 0:T * 8], IDX1[:, off * 8:(off + T) * 8])
            dlc = se2.tile([128, NSMAX], F32, tag="dlce", name="dlce")
            nc.sync.dma_start(dlc[:, 0:NS],
                              DSTLOC[:, slab_off[b]:slab_off[b] + NS])

            g2 = se2.tile([128, TMAX * G2W], F16, tag="g2", name="g2")
            coff = 0
            for s in range(NSCH):
                cl = int(shp[b, s].sum())
                if cl == 0:
                    continue
                for p0 in range(coff, coff + cl, 8192):
                    pl = min(8192, coff + cl - p0)
                    _dma_gather_raw(
                        nc.gpsimd,
                        g2[:, (p0 // 128) * G2W:((p0 + pl) // 128) * G2W]
                        .rearrange("p (t e) -> p t e", e=G2W),
                        tab2[s * SCHW:(s + 1) * SCHW, :],
                        i1[:, p0 // 16:(p0 + pl) // 16], pl, G2W, ROW2,
                        queue_num=s)
                coff += cl

            if clevel < 6:
                nc.vector.memset(lacc[:, b * B * 41:(b + 1) * B * 41], 0.0)
                off += T
                continue
            g23 = g2[:, 0:T * G2W].rearrange("p (t e) -> p t e", e=G2W)
            ga23 = ga2a[:, off * 8:(off + T) * 8] \
                .rearrange("p (t e) -> p t e", e=8)

            at2 = se2.tile([128, TMAX], F16, tag="at2", name="at2")
            at23 = at2[:, 0:T].rearrange("p (t h) -> p t h", h=1)
            nc.gpsimd.tensor_tensor(out=at23, in0=g23[:, :, 41:42],
                                    in1=ga23[:, :, 0:1], op=aadd)
            nc.vector.scalar_tensor_tensor(
                out=at23, in0=at23, scalar=NEG_SLOPE, in1=at23,
                op0=mult, op1=amax)
            w2t = se2.tile([128, TMAX], F32, tag="w2t", name="w2t")
            nc.scalar.activation(out=w2t[:, 0:T], in_=at2[:, 0:T], func=AF.Exp)

            if clevel < 7:
                nc.vector.memset(lacc[:, b * B * 41:(b + 1) * B * 41], 0.0)
                off += T
                continue
            # fused indicator * edge-weight: lhsT carries w2 so g2 stays
            # unweighted (single head), one tensor_scalar per slab
            ind = se2.tile([128, NSMAX * 128], F16, tag="inde", name="inde")
            for i, (j, sl, tl) in enumerate(blk_mm[b]):
                nc.vector.tensor_scalar(
                    out=ind[:, sl * 128:(sl + 1) * 128], in0=iot[:],
                    scalar1=dlc[:, sl:sl + 1], scalar2=w2t[:, tl:tl + 1],
                    op0=eq, op1=mult)

            pse_t = pse.tile([128, B * 41], F32, tag="psE", name="psE")
            nc.vector.memset(pse_t[:], 0.0)
            mm = blk_mm[b]
            last_of = {}
            for i, (j, sl, tl) in enumerate(mm):
                last_of[j] = i
            for i, (j, sl, tl) in enumerate(mm):
                nc.tensor.matmul(
                    pse_t[:, j * 41:(j + 1) * 41],
                    lhsT=ind[:, sl * 128:(sl + 1) * 128],
                    rhs=g2[:, tl * G2W:tl * G2W + 41],
                    start=False, stop=(last_of[j] == i))
            if clevel < 8:
                nc.scalar.copy(lacc[:, b * B * 41:(b + 1) * B * 41],
                               pse_t[:, 0:B * 41])
                off += T
                continue
            # self-loop contribution: slw2 precomputed during phase C
            lsl = lacc[:, b * B * 41:(b + 1) * B * 41]
            nc.vector.tensor_tensor(
                out=lsl, in0=pse_t[:, 0:B * 41],
                in1=shared["slw2"][:, b * B * 41:(b + 1) * B * 41], op=aadd)
            off += T

        # ---- final log_softmax over all slots, one pass ----
        la3 = lacc[:].rearrange("p (j e) -> p j e", e=41)
        rc2 = sbe.tile([128, NSLOT], F32, tag="rc2", name="rc2")
        nc.vector.reciprocal(rc2[:].rearrange("p (j h) -> p j h", h=1),
                             la3[:, :, 40:41])
        lg = sbe.tile([128, NSLOT * 40], F32, tag="lg", name="lg")
        lg3 = lg[:].rearrange("p (j e) -> p j e", e=40)
        nc.vector.tensor_tensor(
            out=lg3, in0=la3[:, :, 0:40],
            in1=rc2[:].rearrange("p (j e) -> p j e", e=1)
            .to_broadcast([128, NSLOT, 40]), op=mult)
        nc.vector.tensor_tensor(
            out=lg3, in0=lg3,
            in1=b2t[:].rearrange("p (j e) -> p j e", j=1)
            .to_broadcast([128, NSLOT, 40]), op=aadd)
        ex = sbe.tile([128, NSLOT * 40], F16, tag="ex", name="ex")
        nc.scalar.activation(out=ex[:], in_=lg[:], func=AF.Exp)
        sm = sbe.tile([128, NSLOT], F32, tag="sm", name="sm")
        nc.vector.reduce_sum(sm[:].rearrange("p (j h) -> p j h", h=1),
                             ex[:].rearrange("p (j e) -> p j e", e=40),
                             axis=AX.X)
        ls = sbe.tile([128, NSLOT], F32, tag="ls", name="ls")
        nc.scalar.activation(out=ls[:], in_=sm[:], func=AF.Ln)
        nc.vector.tensor_tensor(
            out=lg3, in0=lg3,
            in1=ls[:].rearrange("p (j e) -> p j e", e=1)
            .to_broadcast([128, NSLOT, 40]), op=sub)
        nc.sync.dma_start(
            OUT[:].rearrange("(j p) e -> p j e", p=128),
            lg3)


_CACHE = {}


def kernel(x, edge_index, W1, att_src1, att_dst1, b1, W2, att_src2, att_dst2,
           b2):
    in_maps, meta = _prep(np.asarray(x), np.asarray(edge_index),
                          np.asarray(W1), np.asarray(att_src1),
                          np.asarray(att_dst1), np.asarray(W2),
                          np.asarray(att_src2), np.asarray(att_dst2),
                          b1=b1, b2=b2)
    key = meta["shapes"].tobytes()
    if key not in _CACHE:
        _CACHE[key] = _build(meta)
    nc = _CACHE[key]
    res = run_bass_kernel_spmd(nc, in_maps, core_ids=list(range(NCORES)))
    order_ch = meta["order_ch"]
    out = np.empty((N, NCLASS), np.float32)
    for k in range(NCORES):
        ok = np.asarray(res.results[k]["out"])
        for s in range(NSLOT):
            g = int(order_ch[s * NCORES + k])
            r0 = g * DCH
            if r0 >= N:
                continue
            nrow = min(DCH, N - r0)
            out[r0:r0 + nrow] = ok[s * DCH:s * DCH + nrow]
    return out

